# revision 13
# baseline (speedup 1.0000x reference)
"""Trainium2 Bass kernel for the DPAG pairwise-attention + MLP module.

Data-parallel over batch: B=8 batch elements, one per NeuronCore.
Each core computes its full batch row end-to-end on device; the host
only slices inputs per core and stacks the (2,)-outputs back to (8,2).

Math (per batch element, fused — the (Nd,Np,D) intermediate is never
materialized):
    U = concat([smi @ w_att + b_att, gat], 0)          # (145, 64)
    V = pro @ w_att + b_att                            # (1000, 64)
    S[i] = sum_j relu(U[i] + V[j])                     # (145, 64)
    T[j] = sum_i relu(U[i] + V[j])                     # (1000, 64)
    g1 = sigmoid((S/1000) @ w_att + b_att)             # (145, 64)
    g2 = sigmoid((T/145) @ w_att + b_att)              # (1000, 64)
    smi_v = mean_i U[i]*(0.5+g1[i]); pro_v = mean_j pro[j]*(0.5+g2[j])
    out = MLP(concat([smi_v, pro_v]))                  # (2,)

On-chip layout is transposed (D on partitions); the j axis lives in a
permuted-but-consistent order (all j reductions are order-invariant):
j' = 125*n + p  <->  pro row 8p+n.  The pairwise loop splits j between
the Scalar engine (relu+bias with fused row-sum accumulated to PSUM)
and the Vector engine (scalar_tensor_tensor add+max with fused
row-sum); both write fp8 into 512-aligned double-buffered tiles, and
the Tensor engine folds them into T via fp8 DoubleRow identity matmuls
(two loop iterations per matmul).  The MLP middle layers use fp8
DoubleRow as well to halve the PE weight-streaming time.
"""

import numpy as np

import concourse.bacc as bacc
import concourse.mybir as mybir
from concourse import tile
from concourse.tile import add_dep_helper
from concourse.bass_utils import run_bass_kernel_spmd

F32 = mybir.dt.float32
BF16 = mybir.dt.bfloat16
FP8 = mybir.dt.float8e4
AF = mybir.ActivationFunctionType
ALU = mybir.AluOpType
PM = mybir.MatmulPerfMode

B, NS, NA, NP, D = 8, 100, 45, 1000, 64
ND = NS + NA          # 145
NT = (ND + 1) // 2    # 73 pairwise iterations, 2 i-values each
H1, H2, H3, HO = 1024, 1024, 512, 2

# j-axis split between engines in the pairwise loop.
J_ACT = 488                  # scalar engine slice (fp8 out)
J_DVE = NP - J_ACT           # vector engine slice (fp8 out), 512

NEG = -1.0e30


def _ident(nc, ap):
    nc.vector.memset(ap, 0.0)
    nc.gpsimd.affine_select(
        out=ap, in_=ap, compare_op=ALU.not_equal, fill=1.0,
        base=0, pattern=[[-1, ap.shape[-1]]], channel_multiplier=1)


def _build(dbg=False):
    nc = bacc.Bacc("TRN2", target_bir_lowering=False, debug=False)

    smi = nc.dram_tensor("smi", (NS, D), F32, kind="ExternalInput").ap()
    pro = nc.dram_tensor("pro", (NP, D), F32, kind="ExternalInput").ap()
    gat = nc.dram_tensor("gat", (NA, D), F32, kind="ExternalInput").ap()
    w_att = nc.dram_tensor("w_att", (D, D), F32, kind="ExternalInput").ap()
    b_att = nc.dram_tensor("b_att", (D,), F32, kind="ExternalInput").ap()
    w1 = nc.dram_tensor("w1", (2 * D, H1), BF16, kind="ExternalInput").ap()
    b1 = nc.dram_tensor("b1", (H1,), F32, kind="ExternalInput").ap()
    w2 = nc.dram_tensor("w2", (H1, H2), BF16, kind="ExternalInput").ap()
    b2 = nc.dram_tensor("b2", (H2,), F32, kind="ExternalInput").ap()
    w3 = nc.dram_tensor("w3", (H2, H3), BF16, kind="ExternalInput").ap()
    b3 = nc.dram_tensor("b3", (H3,), F32, kind="ExternalInput").ap()
    w4 = nc.dram_tensor("w4", (H3, HO), BF16, kind="ExternalInput").ap()
    b4 = nc.dram_tensor("b4", (HO,), F32, kind="ExternalInput").ap()
    out = nc.dram_tensor("out", (HO,), F32, kind="ExternalOutput").ap()

    dbg_out = {}
    if dbg:
        for name, shape in [
            ("d_U2", (128, 2 * NT)), ("d_PTB", (D, NP)),
            ("d_V2a", (128, J_ACT)), ("d_V2d", (128, J_DVE)),
            ("d_Sa", (128, NT)), ("d_Sd", (128, NT)), ("d_Tsb", (128, NP)),
            ("d_G1", (D, 2 * NT)), ("d_G2", (D, NP)),
            ("d_sv", (D, 1)), ("d_pv", (D, 1)),
        ]:
            dbg_out[name] = nc.dram_tensor(name, shape, F32, kind="ExternalOutput").ap()
    with tile.TileContext(nc) as tc:
        _body(nc, tc, smi, pro, gat, w_att, b_att,
              w1, b1, w2, b2, w3, b3, w4, b4, out, dbg_out)
    nc.compile()
    return nc


def _body(nc, tc, smi, pro, gat, w_att, b_att,
          w1, b1, w2, b2, w3, b3, w4, b4, out, dbg_out=()):
    with (
        tc.tile_pool(name="const", bufs=1) as cp,
        tc.tile_pool(name="ra", bufs=3) as rap,
        tc.tile_pool(name="rd", bufs=3) as rdp,
        tc.tile_pool(name="pst", bufs=1, space="PSUM") as pst,
        tc.tile_pool(name="psw", bufs=2, space="PSUM") as psw,
        tc.tile_pool(name="psh", bufs=1, space="PSUM") as psh,
    ):
        # ---------------- phase-A input DMAs (issued first) -----------
        # pro loaded contiguously: partition p holds pro rows 8p..8p+7
        # (2 KiB per line).  Split into 8 dma_starts by partition range
        # so the runtime spreads them across parallel hardware queues.
        WATT = cp.tile([D, D], F32)
        nc.sync.dma_start(WATT[0:32, :], w_att[0:32, :])
        nc.sync.dma_start(WATT[32:D, :], w_att[32:D, :])
        bdup = cp.tile([128, 1], F32)      # [b_att ; b_att]
        b_col = b_att.rearrange("(d a) -> d a", a=1)
        nc.sync.dma_start(bdup[0:D, :], b_col)
        nc.sync.dma_start(bdup[D:128, :], b_col)
        SMI = cp.tile([NS, D], F32)
        nc.sync.dma_start(SMI[0:50, :], smi[0:50, :])
        nc.sync.dma_start(SMI[50:NS, :], smi[50:NS, :])
        GA2 = cp.tile([NA, 128], F32)
        nc.sync.dma_start(GA2[:, 0:D], gat[:])
        nc.sync.dma_start(GA2[:, D:128], gat[:])
        PROC = cp.tile([125, 512], F32)
        pro_r = pro.rearrange("(p n) d -> p (n d)", p=125)
        pro_dmas = []
        rings = [nc.scalar, nc.sync]
        for c in range(8):
            pp_ = slice(16 * c, min(16 * (c + 1), 125))
            pro_dmas.append(rings[c % 2].dma_start(PROC[pp_, :], pro_r[pp_, :]))

        # identities (built on the vector engine, off the gpsimd queue)
        identb = cp.tile([128, 128], BF16)
        _ident(nc, identb[:])
        ident8 = cp.tile([128, 2, 128], FP8)
        _ident(nc, ident8[:, 0, :])
        _ident(nc, ident8[:, 1, :])
        identf = cp.tile([128, 128], F32)
        _ident(nc, identf[:])

        # Dummy sigmoid first on the ACT queue: forces the activation
        # table set that contains sigmoid+relu+identity+copy, so no
        # mid-kernel table reload happens before the phase-C sigmoids.
        dumm = cp.tile([1, 1], F32)
        nc.vector.memset(dumm[:], 0.0)
        nc.scalar.activation(dumm[:], dumm[:], AF.Sigmoid)

        # w_att stacked forms, built by cheap ACT/DVE casts from WATT
        wdupb = cp.tile([D, 128], BF16)    # [w | w]   -> duplicated M
        nc.scalar.copy(wdupb[:, 0:D], WATT[:])
        nc.vector.tensor_copy(wdupb[:, D:128], WATT[:])
        wstk = cp.tile([128, D], BF16)     # [w ; w]   -> K-stacked (fold)
        zdgb = cp.tile([128, 128], BF16)   # blockdiag(w, w)
        nc.vector.memset(zdgb[:], 0.0)

        # ---------------- weight / constant DMAs ----------------------
        wdmas = []
        W1a = cp.tile([D, H1], BF16)
        W1b = cp.tile([D, H1], BF16)
        wdmas.append(nc.gpsimd.dma_start(W1a[:], w1[0:D, :]))
        wdmas.append(nc.gpsimd.dma_start(W1b[:], w1[D:2 * D, :]))
        # W2/W3 as fp8 in DoubleRow layout: [p, i, c, n] = w[(2c+i)*128+p, n]
        W2 = cp.tile([128, 8, H2], BF16)
        w2r = w2.rearrange("(c p) n -> p c n", p=128)
        for c in range(4):
            wdmas.append(nc.gpsimd.dma_start(W2[:, 2 * c:2 * c + 2, :],
                                             w2r[:, 2 * c:2 * c + 2, :]))
        W3 = cp.tile([128, 8, H3], BF16)
        w3r = w3.rearrange("(c p) n -> p c n", p=128)
        for c in range(2):
            wdmas.append(nc.gpsimd.dma_start(W3[:, 4 * c:4 * c + 4, :],
                                             w3r[:, 4 * c:4 * c + 4, :]))
        W4 = cp.tile([128, 4, HO], BF16)
        wdmas.append(nc.gpsimd.dma_start(W4[:], w4.rearrange("(c p) n -> p c n", p=128)))
        # weights are needed only by the MLP tail; keep them off the wire
        # until the latency-critical pro/smi/gat inputs have landed
        for wd in wdmas:
            add_dep_helper(wd.ins, pro_dmas[-1].ins, sync=True,
                           reason="delay weight DMA behind critical inputs")
        B1sb = cp.tile([128, 8], F32)
        nc.gpsimd.dma_start(B1sb[:], b1.rearrange("(c p) -> p c", p=128))
        B2sb = cp.tile([128, 8], F32)
        nc.gpsimd.dma_start(B2sb[:], b2.rearrange("(c p) -> p c", p=128))
        B3sb = cp.tile([128, 4], F32)
        nc.gpsimd.dma_start(B3sb[:], b3.rearrange("(c p) -> p c", p=128))
        B4sb = cp.tile([HO, 1], F32)
        nc.gpsimd.dma_start(B4sb[:], b4.rearrange("(d a) -> d a", a=1))

        # ---------------- phase A: transposes + projections -----------
        # pro pipeline (all bf16): cast -> 8 PE transposes -> PTB ->
        # 2 projection matmuls -> V2a/V2d with bias.
        PROB = cp.tile([125, 512], BF16)
        for c in range(4):
            cc = slice(128 * c, 128 * (c + 1))
            if c % 2 == 0:
                nc.scalar.copy(PROB[:, cc], PROC[:, cc])
            else:
                nc.vector.tensor_copy(PROB[:, cc], PROC[:, cc])

        PTB = cp.tile([D, NP], BF16)       # pro^T (bf16, permuted j)
        for n in range(8):
            ps = psw.tile([D, 125], BF16, tag="pv")
            nc.tensor.transpose(ps[:], PROB[:, 64 * n:64 * (n + 1)],
                                identb[0:125, 0:125])
            cc = slice(125 * n, 125 * (n + 1))
            if n % 2 == 0:
                nc.scalar.copy(PTB[:, cc], ps[:])
            else:
                nc.vector.tensor_copy(PTB[:, cc], ps[:])

        V2a = cp.tile([128, J_ACT], BF16)  # [pro_att^T;pro_att^T] cols 0:JA
        V2d = cp.tile([128, J_DVE], BF16)  # cols JA:NP
        for h in range(2):
            pv = psw.tile([128, 500], F32, tag="pv")
            nc.tensor.matmul(pv[:], wdupb[:], PTB[:, 500 * h:500 * (h + 1)])
            if h == 0:
                nc.scalar.activation(V2a[:], pv[:, 0:J_ACT],
                                     AF.Identity, bias=bdup[:, 0:1])
                nc.vector.tensor_scalar(V2d[:, 0:500 - J_ACT],
                                        pv[:, J_ACT:500], bdup[:, 0:1],
                                        None, ALU.add)
            else:
                nc.vector.tensor_scalar(V2d[:, 500 - J_ACT:J_DVE],
                                        pv[:], bdup[:, 0:1], None, ALU.add)

        # U2 (128, 146): lower half = U^T columns 0..144, upper half =
        # U^T columns shifted by one; column 145 (and upper 144) are the
        # -1e30 pad so the pair (144,145) contributes relu()=0 for the
        # dummy index.
        U2 = cp.tile([128, 2 * NT], F32)
        nc.vector.memset(U2[:], NEG)
        ps = psw.tile([D, NS], F32, tag="pv")
        nc.tensor.transpose(ps[:], SMI[:], identf[0:NS, 0:NS])
        SMT = cp.tile([D, NS], BF16)
        nc.scalar.copy(SMT[:], ps[:])
        ps = psw.tile([128, NS], F32, tag="pv")
        nc.tensor.matmul(ps[:], wdupb[:], SMT[:])
        nc.scalar.activation(U2[0:D, 0:NS], ps[0:D, :],
                             AF.Identity, bias=bdup[0:D, 0:1])
        nc.scalar.activation(U2[D:128, 0:NS - 1], ps[D:128, 1:NS],
                             AF.Identity, bias=bdup[D:128, 0:1])
        ps = psw.tile([128, NA], F32, tag="pv")
        nc.tensor.matmul(ps[:], GA2[:], identf[0:NA, 0:NA])
        nc.scalar.copy(U2[0:D, NS:ND], ps[0:D, :])
        nc.scalar.copy(U2[D:128, NS - 1:ND - 1], ps[D:128, :])

        # late w_att prep (phase C only): keep it off the critical path
        nc.scalar.copy(wstk[0:D, :], WATT[:])
        nc.vector.tensor_copy(wstk[D:128, :], WATT[:])
        nc.scalar.copy(zdgb[0:D, 0:D], WATT[:])
        nc.vector.tensor_copy(zdgb[D:128, D:128], WATT[:])

        # ---------------- phase B: pairwise relu-sum loop -------------
        # Both engines write fp8 into 512-aligned (128,2,512) ring
        # tiles; PE folds pairs of iterations with DoubleRow matmuls.
        Sa = pst.tile([128, NT], F32, tag="sa")   # ACT row-sums (PSUM)
        Sd = cp.tile([128, NT], F32)              # DVE row-sums
        TA = pst.tile([128, J_ACT], F32, tag="ta")
        TD = pst.tile([128, J_DVE], F32, tag="td")
        Zz = cp.tile([128, J_DVE], BF16)
        nc.vector.memset(Zz[:], 0.0)
        RD = None
        for t in range(NT):
            q, ph = divmod(t, 2)
            u_col = U2[:, 2 * t:2 * t + 1]
            if ph == 0:
                RD = rdp.tile([128, 2, 512], FP8, tag="rd")
            RA = rap.tile([128, J_ACT], BF16, tag="ra")
            nc.scalar.activation(RA[:], V2a[:], AF.Relu,
                                 bias=u_col, accum_out=Sa[:, t:t + 1])
            nc.vector.scalar_tensor_tensor(RD[:, ph, 0:J_DVE], V2d[:],
                                           u_col, Zz[:], ALU.add, ALU.max,
                                           accum_out=Sd[:, t:t + 1])
            nc.tensor.matmul(TA[:], identb[:], RA[:],
                             start=(t == 0), stop=(t == NT - 1))
            if ph == 1:
                nc.tensor.matmul(TD[:], ident8[:], RD[:, :, 0:J_DVE],
                                 perf_mode=PM.DoubleRow,
                                 start=(q == 0), stop=False)
            elif t == NT - 1:
                nc.tensor.matmul(TD[:], ident8[:, 0, :], RD[:, 0, 0:J_DVE],
                                 start=False, stop=True)

        # ---------------- phase C: gates + pooled vectors -------------
        # T (fold of upper/lower halves) -> g2, via K=128 matmul with
        # the K-stacked w_att.  Interleave copy/matmul/sigmoid/pool per
        # 250-column chunk so the chain starts right at loop end.
        Tsb = cp.tile([128, NP], BF16)
        G2 = cp.tile([D, NP], F32)
        pp = cp.tile([D, NP], F32)
        sp4 = cp.tile([D, 4], F32)
        for c in range(4):
            cc = slice(250 * c, 250 * (c + 1))
            if c == 0:
                nc.scalar.copy(Tsb[:, 0:250], TA[:, 0:250])
            elif c == 1:
                nc.vector.tensor_copy(Tsb[:, 250:J_ACT], TA[:, 250:J_ACT])
                nc.scalar.copy(Tsb[:, J_ACT:500], TD[:, 0:12])
            elif c == 2:
                nc.scalar.copy(Tsb[:, 500:750], TD[:, 12:262])
            else:
                nc.vector.tensor_copy(Tsb[:, 750:NP], TD[:, 262:J_DVE])
            ps = psw.tile([D, 250], F32, tag="pv")
            nc.tensor.matmul(ps[:], wstk[:], Tsb[:, cc])
            nc.scalar.activation(G2[:, cc], ps[:], AF.Sigmoid,
                                 bias=bdup[0:D, 0:1], scale=1.0 / ND)
            nc.vector.scalar_tensor_tensor(pp[:, cc], G2[:, cc], 0.5,
                                           PTB[:, cc],
                                           ALU.add, ALU.mult,
                                           accum_out=sp4[:, c:c + 1])

        # S2 = Sa + Sd (bf16); g1 halves via blockdiag(w,w) matmuls.
        S2b = cp.tile([128, NT], BF16)
        nc.vector.tensor_tensor(S2b[:], Sa[:], Sd[:], ALU.add)
        psm = psw.tile([D, 2 * NT], F32, tag="pv")
        nc.tensor.matmul(psm[:, 0:NT], zdgb[:, 0:D], S2b[:])
        nc.tensor.matmul(psm[:, NT:2 * NT], zdgb[:, D:128], S2b[:])
        G1 = cp.tile([D, 2 * NT], F32)
        nc.scalar.activation(G1[:], psm[:], AF.Sigmoid,
                             bias=bdup[0:D, 0:1], scale=1.0 / NP)

        # pooled vectors
        pe = cp.tile([D, NT], F32)
        po = cp.tile([D, NT - 1], F32)
        se = cp.tile([D, 1], F32)
        so = cp.tile([D, 1], F32)
        sp_ = cp.tile([D, 1], F32)
        nc.vector.scalar_tensor_tensor(pe[:], G1[:, 0:NT], 0.5,
                                       U2[0:D, 0:2 * NT - 1:2],
                                       ALU.add, ALU.mult, accum_out=se[:])
        nc.vector.scalar_tensor_tensor(po[:], G1[:, NT:2 * NT - 1], 0.5,
                                       U2[0:D, 1:2 * NT - 2:2],
                                       ALU.add, ALU.mult, accum_out=so[:])
        nc.vector.tensor_reduce(sp_[:], sp4[:], mybir.AxisListType.X, ALU.add)
        sv = cp.tile([D, 1], F32)
        nc.vector.tensor_tensor(sv[:], se[:], so[:], ALU.add)
        smi_v = cp.tile([D, 1], F32)
        nc.scalar.mul(smi_v[:], sv[:], 1.0 / ND)
        pro_v = cp.tile([D, 1], F32)
        nc.scalar.mul(pro_v[:], sp_[:], 1.0 / NP)

        # ---------------- phase D: MLP head -------------------------
        smi_vb = cp.tile([D, 1], BF16)
        nc.scalar.copy(smi_vb[:], smi_v[:])
        pro_vb = cp.tile([D, 1], BF16)
        nc.scalar.copy(pro_vb[:], pro_v[:])

        ph1 = psh.tile([128, 8], F32, tag="h")
        for m in range(8):
            mm = slice(128 * m, 128 * (m + 1))
            nc.tensor.matmul(ph1[:, m:m + 1], W1a[:, mm], smi_vb[:],
                             start=True, stop=False)
            nc.tensor.matmul(ph1[:, m:m + 1], W1b[:, mm], pro_vb[:],
                             start=False, stop=True)
        nc.vector.tensor_tensor(ph1[:], ph1[:], B1sb[:], ALU.add)
        Ht1 = cp.tile([128, 8], BF16)
        nc.scalar.activation(Ht1[:], ph1[:], AF.Relu)

        ph2 = psh.tile([128, 8], F32, tag="h")
        for m in range(8):
            mm = slice(128 * m, 128 * (m + 1))
            for c in range(8):
                nc.tensor.matmul(ph2[:, m:m + 1], W2[:, c, mm], Ht1[:, c:c + 1],
                                 start=(c == 0), stop=(c == 7))
        nc.vector.tensor_tensor(ph2[:], ph2[:], B2sb[:], ALU.add)
        Ht2 = cp.tile([128, 8], BF16)
        nc.scalar.activation(Ht2[:], ph2[:], AF.Relu)

        ph3 = psh.tile([128, 4], F32, tag="h3")
        for m in range(4):
            mm = slice(128 * m, 128 * (m + 1))
            for c in range(8):
                nc.tensor.matmul(ph3[:, m:m + 1], W3[:, c, mm], Ht2[:, c:c + 1],
                                 start=(c == 0), stop=(c == 7))
        nc.vector.tensor_tensor(ph3[:], ph3[:], B3sb[:], ALU.add)
        Ht3 = cp.tile([128, 4], BF16)
        nc.scalar.activation(Ht3[:], ph3[:], AF.Relu)

        ph4 = psh.tile([HO, 1], F32, tag="h4")
        for c in range(4):
            nc.tensor.matmul(ph4[:], W4[:, c, :], Ht3[:, c:c + 1],
                             start=(c == 0), stop=(c == 3))
        nc.vector.tensor_tensor(ph4[:], ph4[:], B4sb[:], ALU.add)
        osb = cp.tile([HO, 1], F32)
        nc.scalar.copy(osb[:], ph4[:])
        nc.sync.dma_start(out.rearrange("(a b) -> a b", b=1), osb[:])

        if dbg_out:
            SaC = cp.tile([128, NT], F32)
            nc.scalar.copy(SaC[:], Sa[:])
            Sa = SaC
            for name, tl in [("d_U2", U2), ("d_PTB", PTB),
                             ("d_V2a", V2a), ("d_V2d", V2d),
                             ("d_Sa", Sa), ("d_Sd", Sd), ("d_Tsb", Tsb),
                             ("d_G1", G1), ("d_G2", G2),
                             ("d_sv", smi_v), ("d_pv", pro_v)]:
                nc.gpsimd.dma_start(dbg_out[name], tl[:])


_NC = None


def kernel(smi_tf, pro_tf, drug_gat, w_att, b_att,
           w1, b1, w2, b2, w3, b3, w4, b4):
    global _NC
    if _NC is None:
        _NC = _build()
    import ml_dtypes
    f32 = lambda a: np.ascontiguousarray(np.asarray(a), dtype=np.float32)
    bf16 = lambda a: np.ascontiguousarray(np.asarray(a), dtype=ml_dtypes.bfloat16)
    fp8 = lambda a: np.ascontiguousarray(np.asarray(a), dtype=mybir.dt.np(FP8))
    shared = {
        "w_att": f32(w_att), "b_att": f32(b_att),
        "w1": bf16(w1), "b1": f32(b1), "w2": bf16(w2), "b2": f32(b2),
        "w3": bf16(w3), "b3": f32(b3), "w4": bf16(w4), "b4": f32(b4),
    }
    in_maps = [
        {"smi": f32(smi_tf[b]), "pro": f32(pro_tf[b]),
         "gat": f32(drug_gat[b]), **shared}
        for b in range(B)
    ]
    res = run_bass_kernel_spmd(_NC, in_maps, core_ids=list(range(B)))
    return np.stack([res.results[b]["out"] for b in range(B)], axis=0)


# revision 17
# speedup vs baseline: 1.0755x; 1.0755x over previous
"""Trainium2 Bass kernel for the DPAG pairwise-attention + MLP module.

Data-parallel over batch: B=8 batch elements, one per NeuronCore.
Each core computes its full batch row end-to-end on device; the host
only slices inputs per core and stacks the (2,)-outputs back to (8,2).

Math (per batch element, fused — the (Nd,Np,D) intermediate is never
materialized):
    U = concat([smi @ w_att + b_att, gat], 0)          # (145, 64)
    V = pro @ w_att + b_att                            # (1000, 64)
    S[i] = sum_j relu(U[i] + V[j])                     # (145, 64)
    T[j] = sum_i relu(U[i] + V[j])                     # (1000, 64)
    g1 = sigmoid((S/1000) @ w_att + b_att)             # (145, 64)
    g2 = sigmoid((T/145) @ w_att + b_att)              # (1000, 64)
    smi_v = mean_i U[i]*(0.5+g1[i]); pro_v = mean_j pro[j]*(0.5+g2[j])
    out = MLP(concat([smi_v, pro_v]))                  # (2,)

On-chip layout is transposed (D on partitions); the j axis lives in a
permuted-but-consistent order (all j reductions are order-invariant):
j' = 125*n + p  <->  pro row 8p+n.  The pairwise loop splits j between
the Scalar engine (relu+bias with fused row-sum accumulated to PSUM)
and the Vector engine (scalar_tensor_tensor add+max with fused
row-sum); both write fp8 into 512-aligned double-buffered tiles, and
the Tensor engine folds them into T via fp8 DoubleRow identity matmuls
(two loop iterations per matmul).  The MLP middle layers use fp8
DoubleRow as well to halve the PE weight-streaming time.
"""

import numpy as np

import concourse.bacc as bacc
import concourse.mybir as mybir
from concourse import tile
from concourse.tile import add_dep_helper
from concourse.bass_utils import run_bass_kernel_spmd

F32 = mybir.dt.float32
BF16 = mybir.dt.bfloat16
FP8 = mybir.dt.float8e4
AF = mybir.ActivationFunctionType
ALU = mybir.AluOpType
PM = mybir.MatmulPerfMode

B, NS, NA, NP, D = 8, 100, 45, 1000, 64
ND = NS + NA          # 145
NT = (ND + 1) // 2    # 73 pairwise iterations, 2 i-values each
H1, H2, H3, HO = 1024, 1024, 512, 2

# j-axis split between engines in the pairwise loop.
J_ACT = 360                  # scalar engine slice (bf16 out)
J_DVE = NP - J_ACT           # vector engine slice (fp8 out), 640
J_H = 512                    # TD bank split point (aligned)

NEG = -1.0e30


def _ident(nc, ap):
    nc.vector.memset(ap, 0.0)
    nc.gpsimd.affine_select(
        out=ap, in_=ap, compare_op=ALU.not_equal, fill=1.0,
        base=0, pattern=[[-1, ap.shape[-1]]], channel_multiplier=1)


def _build(dbg=False):
    nc = bacc.Bacc("TRN2", target_bir_lowering=False, debug=False)

    smi = nc.dram_tensor("smi", (NS, D), F32, kind="ExternalInput").ap()
    pro = nc.dram_tensor("pro", (NP, D), F32, kind="ExternalInput").ap()
    gat = nc.dram_tensor("gat", (NA, D), F32, kind="ExternalInput").ap()
    w_att = nc.dram_tensor("w_att", (D, D), F32, kind="ExternalInput").ap()
    b_att = nc.dram_tensor("b_att", (D,), F32, kind="ExternalInput").ap()
    w1 = nc.dram_tensor("w1", (2 * D, H1), BF16, kind="ExternalInput").ap()
    b1 = nc.dram_tensor("b1", (H1,), F32, kind="ExternalInput").ap()
    w2 = nc.dram_tensor("w2", (H1, H2), BF16, kind="ExternalInput").ap()
    b2 = nc.dram_tensor("b2", (H2,), F32, kind="ExternalInput").ap()
    w3 = nc.dram_tensor("w3", (H2, H3), BF16, kind="ExternalInput").ap()
    b3 = nc.dram_tensor("b3", (H3,), F32, kind="ExternalInput").ap()
    w4 = nc.dram_tensor("w4", (H3, HO), BF16, kind="ExternalInput").ap()
    b4 = nc.dram_tensor("b4", (HO,), F32, kind="ExternalInput").ap()
    out = nc.dram_tensor("out", (HO,), F32, kind="ExternalOutput").ap()

    dbg_out = {}
    if dbg:
        for name, shape in [
            ("d_U2", (128, 2 * NT)), ("d_PTB", (D, NP)),
            ("d_V2a", (128, J_ACT)), ("d_V2d", (128, J_DVE)),
            ("d_Sa", (128, NT)), ("d_Sd", (128, NT)), ("d_Tsb", (128, NP)),
            ("d_G1", (D, 2 * NT)), ("d_G2", (D, NP)),
            ("d_sv", (D, 1)), ("d_pv", (D, 1)),
        ]:
            dbg_out[name] = nc.dram_tensor(name, shape, F32, kind="ExternalOutput").ap()
    with tile.TileContext(nc) as tc:
        _body(nc, tc, smi, pro, gat, w_att, b_att,
              w1, b1, w2, b2, w3, b3, w4, b4, out, dbg_out)
    nc.compile()
    return nc


def _body(nc, tc, smi, pro, gat, w_att, b_att,
          w1, b1, w2, b2, w3, b3, w4, b4, out, dbg_out=()):
    with (
        tc.tile_pool(name="const", bufs=1) as cp,
        tc.tile_pool(name="ra", bufs=3) as rap,
        tc.tile_pool(name="rd", bufs=3) as rdp,
        tc.tile_pool(name="pst", bufs=1, space="PSUM") as pst,
        tc.tile_pool(name="psw", bufs=2, space="PSUM") as psw,
        tc.tile_pool(name="psh", bufs=1, space="PSUM") as psh,
    ):
        # ---------------- phase-A input DMAs (issued first) -----------
        # pro loaded contiguously: partition p holds pro rows 8p..8p+7
        # (2 KiB per line).  Split into 8 dma_starts by partition range
        # so the runtime spreads them across parallel hardware queues.
        PROC = cp.tile([125, 512], F32)
        pro_r = pro.rearrange("(p n) d -> p (n d)", p=125)
        pro_dmas = []
        for c in range(4):
            pp_ = slice(32 * c, min(32 * (c + 1), 125))
            pro_dmas.append(nc.sync.dma_start(PROC[pp_, :], pro_r[pp_, :]))
        WATT = cp.tile([D, D], F32)
        nc.sync.dma_start(WATT[0:32, :], w_att[0:32, :])
        nc.sync.dma_start(WATT[32:D, :], w_att[32:D, :])
        bdup = cp.tile([128, 1], F32)      # [b_att ; b_att]
        b_col = b_att.rearrange("(d a) -> d a", a=1)
        nc.sync.dma_start(bdup[0:D, :], b_col)
        nc.sync.dma_start(bdup[D:128, :], b_col)
        SMI = cp.tile([NS, D], F32)
        nc.scalar.dma_start(SMI[0:50, :], smi[0:50, :])
        nc.scalar.dma_start(SMI[50:NS, :], smi[50:NS, :])
        GA2 = cp.tile([NA, 128], F32)
        nc.scalar.dma_start(GA2[:, 0:D], gat[:])
        nc.scalar.dma_start(GA2[:, D:128], gat[:])

        # identities (built on the vector engine, off the gpsimd queue)
        identb = cp.tile([128, 128], BF16)
        _ident(nc, identb[:])
        ident8 = cp.tile([128, 2, 128], FP8)
        _ident(nc, ident8[:, 0, :])
        _ident(nc, ident8[:, 1, :])
        identf = cp.tile([128, 128], F32)
        _ident(nc, identf[:])

        # w_att stacked forms, built by cheap ACT/DVE casts from WATT
        wdupb = cp.tile([D, 128], BF16)    # [w | w]   -> duplicated M
        nc.scalar.copy(wdupb[:, 0:D], WATT[:])
        nc.vector.tensor_copy(wdupb[:, D:128], WATT[:])
        wstk = cp.tile([128, D], BF16)     # [w ; w]   -> K-stacked (fold)
        zdgb = cp.tile([128, 128], BF16)   # blockdiag(w, w)
        nc.vector.memset(zdgb[:], 0.0)

        # ---------------- weight / constant DMAs ----------------------
        wdmas = []
        W1a = cp.tile([D, H1], BF16)
        W1b = cp.tile([D, H1], BF16)
        wdmas.append(nc.gpsimd.dma_start(W1a[:], w1[0:D, :]))
        wdmas.append(nc.gpsimd.dma_start(W1b[:], w1[D:2 * D, :]))
        # W2/W3 as fp8 in DoubleRow layout: [p, i, c, n] = w[(2c+i)*128+p, n]
        W2 = cp.tile([128, 8, H2], BF16)
        w2r = w2.rearrange("(c p) n -> p c n", p=128)
        for c in range(4):
            wdmas.append(nc.gpsimd.dma_start(W2[:, 2 * c:2 * c + 2, :],
                                             w2r[:, 2 * c:2 * c + 2, :]))
        W3 = cp.tile([128, 8, H3], BF16)
        w3r = w3.rearrange("(c p) n -> p c n", p=128)
        for c in range(2):
            wdmas.append(nc.gpsimd.dma_start(W3[:, 4 * c:4 * c + 4, :],
                                             w3r[:, 4 * c:4 * c + 4, :]))
        W4 = cp.tile([128, 4, HO], BF16)
        wdmas.append(nc.gpsimd.dma_start(W4[:], w4.rearrange("(c p) n -> p c n", p=128)))
        # weights are needed only by the MLP tail; keep them off the wire
        # until the latency-critical pro/smi/gat inputs have landed
        for wd in wdmas:
            add_dep_helper(wd.ins, pro_dmas[-1].ins, sync=True,
                           reason="delay weight DMA behind critical inputs")
        B1sb = cp.tile([128, 8], F32)
        nc.gpsimd.dma_start(B1sb[:], b1.rearrange("(c p) -> p c", p=128))
        B2sb = cp.tile([128, 8], F32)
        nc.gpsimd.dma_start(B2sb[:], b2.rearrange("(c p) -> p c", p=128))
        B3sb = cp.tile([128, 4], F32)
        nc.gpsimd.dma_start(B3sb[:], b3.rearrange("(c p) -> p c", p=128))
        B4sb = cp.tile([HO, 1], F32)
        nc.gpsimd.dma_start(B4sb[:], b4.rearrange("(d a) -> d a", a=1))

        # ---------------- phase A: transposes + projections -----------
        # pro pipeline (all bf16): cast -> 8 PE transposes -> PTB ->
        # 2 projection matmuls -> V2a/V2d with bias.
        PROB = cp.tile([125, 512], BF16)
        for c in range(4):
            cc = slice(128 * c, 128 * (c + 1))
            if c % 2 == 0:
                nc.scalar.copy(PROB[:, cc], PROC[:, cc])
            else:
                nc.vector.tensor_copy(PROB[:, cc], PROC[:, cc])

        PTB = cp.tile([D, NP], BF16)       # pro^T (bf16, permuted j)
        for n in range(8):
            ps = psw.tile([D, 125], BF16, tag="pv")
            nc.tensor.transpose(ps[:], PROB[:, 64 * n:64 * (n + 1)],
                                identb[0:125, 0:125])
            cc = slice(125 * n, 125 * (n + 1))
            if n % 2 == 0:
                nc.scalar.copy(PTB[:, cc], ps[:])
            else:
                nc.vector.tensor_copy(PTB[:, cc], ps[:])

        V2a = cp.tile([128, J_ACT], BF16)  # [pro_att^T;pro_att^T] cols 0:JA
        V2d = cp.tile([128, J_DVE], BF16)  # cols JA:NP
        for h in range(2):
            pv = psw.tile([128, 500], F32, tag="pv")
            nc.tensor.matmul(pv[:], wdupb[:], PTB[:, 500 * h:500 * (h + 1)])
            if h == 0:
                nc.scalar.activation(V2a[:], pv[:, 0:J_ACT],
                                     AF.Identity, bias=bdup[:, 0:1])
                nc.vector.tensor_scalar(V2d[:, 0:500 - J_ACT],
                                        pv[:, J_ACT:500], bdup[:, 0:1],
                                        None, ALU.add)
            else:
                nc.vector.tensor_scalar(V2d[:, 500 - J_ACT:J_DVE],
                                        pv[:], bdup[:, 0:1], None, ALU.add)

        # U2 (128, 146): lower half = U^T columns 0..144, upper half =
        # U^T columns shifted by one; column 145 (and upper 144) are the
        # -1e30 pad so the pair (144,145) contributes relu()=0 for the
        # dummy index.
        U2 = cp.tile([128, 2 * NT], F32)
        nc.vector.memset(U2[:], NEG)
        ps = psw.tile([D, NS], F32, tag="pv")
        nc.tensor.transpose(ps[:], SMI[:], identf[0:NS, 0:NS])
        SMT = cp.tile([D, NS], BF16)
        nc.scalar.copy(SMT[:], ps[:])
        ps = psw.tile([128, NS], F32, tag="pv")
        nc.tensor.matmul(ps[:], wdupb[:], SMT[:])
        nc.scalar.activation(U2[0:D, 0:NS], ps[0:D, :],
                             AF.Identity, bias=bdup[0:D, 0:1])
        nc.scalar.activation(U2[D:128, 0:NS - 1], ps[D:128, 1:NS],
                             AF.Identity, bias=bdup[D:128, 0:1])
        ps = psw.tile([128, NA], F32, tag="pv")
        nc.tensor.matmul(ps[:], GA2[:], identf[0:NA, 0:NA])
        nc.scalar.copy(U2[0:D, NS:ND], ps[0:D, :])
        nc.scalar.copy(U2[D:128, NS - 1:ND - 1], ps[D:128, :])

        # late w_att prep (phase C only): keep it off the critical path
        nc.scalar.copy(wstk[0:D, :], WATT[:])
        nc.vector.tensor_copy(wstk[D:128, :], WATT[:])
        nc.scalar.copy(zdgb[0:D, 0:D], WATT[:])
        nc.vector.tensor_copy(zdgb[D:128, D:128], WATT[:])

        # ---------------- phase B: pairwise relu-sum loop -------------
        # Both engines write fp8 into 512-aligned (128,2,512) ring
        # tiles; PE folds pairs of iterations with DoubleRow matmuls.
        Sa = pst.tile([128, NT], F32, tag="sa")   # ACT row-sums (PSUM)
        Sd = cp.tile([128, NT], F32)              # DVE row-sums
        TA = pst.tile([128, J_ACT], F32, tag="ta")
        TD1 = pst.tile([128, J_H], F32, tag="td1")
        TD2 = pst.tile([128, J_DVE - J_H], F32, tag="td2")
        Zz = cp.tile([128, J_DVE], BF16)
        nc.vector.memset(Zz[:], 0.0)
        RD = None
        for t in range(NT):
            q, ph = divmod(t, 2)
            u_col = U2[:, 2 * t:2 * t + 1]
            if ph == 0:
                RD = rdp.tile([128, 2, 1024], FP8, tag="rd")
            RA = rap.tile([128, J_ACT], BF16, tag="ra")
            nc.scalar.activation(RA[:], V2a[:], AF.Relu,
                                 bias=u_col, accum_out=Sa[:, t:t + 1])
            nc.vector.scalar_tensor_tensor(RD[:, ph, 0:J_DVE], V2d[:],
                                           u_col, Zz[:], ALU.add, ALU.max,
                                           accum_out=Sd[:, t:t + 1])
            nc.tensor.matmul(TA[:], identb[:], RA[:],
                             start=(t == 0), stop=(t == NT - 1))
            if ph == 1:
                nc.tensor.matmul(TD1[:], ident8[:], RD[:, :, 0:J_H],
                                 perf_mode=PM.DoubleRow,
                                 start=(q == 0), stop=False)
                nc.tensor.matmul(TD2[:], ident8[:], RD[:, :, J_H:J_DVE],
                                 perf_mode=PM.DoubleRow,
                                 start=(q == 0), stop=False)
            elif t == NT - 1:
                nc.tensor.matmul(TD1[:], ident8[:, 0, :], RD[:, 0, 0:J_H],
                                 start=False, stop=True)
                nc.tensor.matmul(TD2[:], ident8[:, 0, :], RD[:, 0, J_H:J_DVE],
                                 start=False, stop=True)

        # ---------------- phase C: gates + pooled vectors -------------
        # T (fold of upper/lower halves) -> g2, via K=128 matmul with
        # the K-stacked w_att.  Interleave copy/matmul/sigmoid/pool per
        # 250-column chunk so the chain starts right at loop end.
        Tsb = cp.tile([128, NP], BF16)
        G2 = cp.tile([D, NP], F32)
        pp = cp.tile([D, NP], F32)
        sp4 = cp.tile([D, 4], F32)
        for c in range(4):
            cc = slice(250 * c, 250 * (c + 1))
            if c == 0:
                nc.scalar.copy(Tsb[:, 0:250], TA[:, 0:250])
            elif c == 1:
                nc.vector.tensor_copy(Tsb[:, 250:J_ACT], TA[:, 250:J_ACT])
                nc.scalar.copy(Tsb[:, J_ACT:500], TD1[:, 0:500 - J_ACT])
            elif c == 2:
                nc.scalar.copy(Tsb[:, 500:750], TD1[:, 500 - J_ACT:750 - J_ACT])
            else:
                nc.vector.tensor_copy(Tsb[:, 750:J_ACT + J_H],
                                      TD1[:, 750 - J_ACT:J_H])
                nc.vector.tensor_copy(Tsb[:, J_ACT + J_H:NP],
                                      TD2[:, 0:NP - J_ACT - J_H])
            ps = psw.tile([D, 250], F32, tag="pv")
            nc.tensor.matmul(ps[:], wstk[:], Tsb[:, cc])
            nc.scalar.activation(G2[:, cc], ps[:], AF.Sigmoid,
                                 bias=bdup[0:D, 0:1], scale=1.0 / ND)
            nc.vector.scalar_tensor_tensor(pp[:, cc], G2[:, cc], 0.5,
                                           PTB[:, cc],
                                           ALU.add, ALU.mult,
                                           accum_out=sp4[:, c:c + 1])

        # S2 = Sa + Sd (bf16); g1 halves via blockdiag(w,w) matmuls.
        S2b = cp.tile([128, NT], BF16)
        nc.vector.tensor_tensor(S2b[:], Sa[:], Sd[:], ALU.add)
        psm = psw.tile([D, 2 * NT], F32, tag="pv")
        nc.tensor.matmul(psm[:, 0:NT], zdgb[:, 0:D], S2b[:])
        nc.tensor.matmul(psm[:, NT:2 * NT], zdgb[:, D:128], S2b[:])
        G1 = cp.tile([D, 2 * NT], F32)
        nc.scalar.activation(G1[:], psm[:], AF.Sigmoid,
                             bias=bdup[0:D, 0:1], scale=1.0 / NP)

        # pooled vectors
        pe = cp.tile([D, NT], F32)
        po = cp.tile([D, NT - 1], F32)
        se = cp.tile([D, 1], F32)
        so = cp.tile([D, 1], F32)
        sp_ = cp.tile([D, 1], F32)
        nc.vector.scalar_tensor_tensor(pe[:], G1[:, 0:NT], 0.5,
                                       U2[0:D, 0:2 * NT - 1:2],
                                       ALU.add, ALU.mult, accum_out=se[:])
        nc.vector.scalar_tensor_tensor(po[:], G1[:, NT:2 * NT - 1], 0.5,
                                       U2[0:D, 1:2 * NT - 2:2],
                                       ALU.add, ALU.mult, accum_out=so[:])
        nc.vector.tensor_reduce(sp_[:], sp4[:], mybir.AxisListType.X, ALU.add)
        sv = cp.tile([D, 1], F32)
        nc.vector.tensor_tensor(sv[:], se[:], so[:], ALU.add)
        smi_v = cp.tile([D, 1], F32)
        nc.scalar.mul(smi_v[:], sv[:], 1.0 / ND)
        pro_v = cp.tile([D, 1], F32)
        nc.scalar.mul(pro_v[:], sp_[:], 1.0 / NP)

        # ---------------- phase D: MLP head -------------------------
        smi_vb = cp.tile([D, 1], BF16)
        nc.scalar.copy(smi_vb[:], smi_v[:])
        pro_vb = cp.tile([D, 1], BF16)
        nc.scalar.copy(pro_vb[:], pro_v[:])

        ph1 = psh.tile([128, 8], F32, tag="h")
        for m in range(8):
            mm = slice(128 * m, 128 * (m + 1))
            nc.tensor.matmul(ph1[:, m:m + 1], W1a[:, mm], smi_vb[:],
                             start=True, stop=False)
            nc.tensor.matmul(ph1[:, m:m + 1], W1b[:, mm], pro_vb[:],
                             start=False, stop=True)
        nc.vector.tensor_tensor(ph1[:], ph1[:], B1sb[:], ALU.add)
        Ht1 = cp.tile([128, 8], BF16)
        nc.scalar.activation(Ht1[:], ph1[:], AF.Relu)

        ph2 = psh.tile([128, 8], F32, tag="h3")
        for m in range(8):
            mm = slice(128 * m, 128 * (m + 1))
            for c in range(8):
                nc.tensor.matmul(ph2[:, m:m + 1], W2[:, c, mm], Ht1[:, c:c + 1],
                                 start=(c == 0), stop=(c == 7))
        nc.vector.tensor_tensor(ph2[:], ph2[:], B2sb[:], ALU.add)
        Ht2 = cp.tile([128, 8], BF16)
        nc.scalar.activation(Ht2[:], ph2[:], AF.Relu)

        ph3f = psh.tile([128, 8], F32, tag="h")
        ph3 = ph3f[:, 0:4]
        for m in range(4):
            mm = slice(128 * m, 128 * (m + 1))
            for c in range(8):
                nc.tensor.matmul(ph3[:, m:m + 1], W3[:, c, mm], Ht2[:, c:c + 1],
                                 start=(c == 0), stop=(c == 7))
        nc.vector.tensor_tensor(ph3, ph3, B3sb[:], ALU.add)
        Ht3 = cp.tile([128, 4], BF16)
        nc.scalar.activation(Ht3[:], ph3, AF.Relu)

        ph4f = psh.tile([128, 8], F32, tag="h3")
        ph4 = ph4f[0:HO, 0:1]
        for c in range(4):
            nc.tensor.matmul(ph4, W4[:, c, :], Ht3[:, c:c + 1],
                             start=(c == 0), stop=(c == 3))
        nc.vector.tensor_tensor(ph4, ph4, B4sb[:], ALU.add)
        osb = cp.tile([HO, 1], F32)
        nc.scalar.copy(osb[:], ph4)
        nc.sync.dma_start(out.rearrange("(a b) -> a b", b=1), osb[:])

        if dbg_out:
            SaC = cp.tile([128, NT], F32)
            nc.scalar.copy(SaC[:], Sa[:])
            Sa = SaC
            for name, tl in [("d_U2", U2), ("d_PTB", PTB),
                             ("d_V2a", V2a), ("d_V2d", V2d),
                             ("d_Sa", Sa), ("d_Sd", Sd), ("d_Tsb", Tsb),
                             ("d_G1", G1), ("d_G2", G2),
                             ("d_sv", smi_v), ("d_pv", pro_v)]:
                nc.gpsimd.dma_start(dbg_out[name], tl[:])


_NC = None


def kernel(smi_tf, pro_tf, drug_gat, w_att, b_att,
           w1, b1, w2, b2, w3, b3, w4, b4):
    global _NC
    if _NC is None:
        _NC = _build()
    import ml_dtypes
    f32 = lambda a: np.ascontiguousarray(np.asarray(a), dtype=np.float32)
    bf16 = lambda a: np.ascontiguousarray(np.asarray(a), dtype=ml_dtypes.bfloat16)
    fp8 = lambda a: np.ascontiguousarray(np.asarray(a), dtype=mybir.dt.np(FP8))
    shared = {
        "w_att": f32(w_att), "b_att": f32(b_att),
        "w1": bf16(w1), "b1": f32(b1), "w2": bf16(w2), "b2": f32(b2),
        "w3": bf16(w3), "b3": f32(b3), "w4": bf16(w4), "b4": f32(b4),
    }
    in_maps = [
        {"smi": f32(smi_tf[b]), "pro": f32(pro_tf[b]),
         "gat": f32(drug_gat[b]), **shared}
        for b in range(B)
    ]
    res = run_bass_kernel_spmd(_NC, in_maps, core_ids=list(range(B)))
    return np.stack([res.results[b]["out"] for b in range(B)], axis=0)
